# revision 1
# baseline (speedup 1.0000x reference)
"""MoE LoRA delta kernel for Trainium2 (Bass/Tile), 8-core SPMD.

Computation (reference):
  flat [T,F] -> logits = flat @ Wr.T [T,E]; top-2 softmax gates
  mid = flat @ A[e].T  [T,E,R];  delta = sum_e gates[:,e] * (mid[e] @ B[e].T) * SCALE

Shapes: T=4096 (2x2048), F=O=4096, E=4, R=16, SCALE=4.0.

Strategy:
  - Token-shard T across 8 cores (512 tokens each); replicate tiny weights.
  - Fold router Wr into the first matmul: W1 = [A_cat; Wr] -> [68, 4096];
    mm1 computes midT_ext [68, 512] = W1 @ x_c.T per core.
  - x.T obtained via PE transpose (128x128 tiles, fp32 exact).
  - Gates: PE-transpose logits to [t,4], top-2 softmax via exp/masking on
    DVE/ACT, transpose back, replicate [4->64] rows via a 0/1 matmul.
  - mm2: delta [128t, 512o] tiles = (gates*midT).T-chunks @ B_cat.T, fp32r.
  - fp32r used for the two big matmuls (full PE rate at N>=512);
    SCALE folded into B host-side (power of two -> exact).
"""

import os
import numpy as np

import concourse.bass as bass
import concourse.mybir as mybir
import concourse.tile as tile
from concourse import bacc, masks
from concourse.bass_utils import run_bass_kernel_spmd

F32 = mybir.dt.float32
F32R = mybir.dt.float32r

N_CORES = 8
T_FULL = 4096          # 2*2048 tokens
TC = T_FULL // N_CORES  # 512 tokens per core
F = 4096
O = 4096
E = 4
R = 16
ER = E * R             # 64
M1 = ER + E            # 68 (A rows + router rows)
KB = F // 128          # 32 f-blocks
NT = TC // 128         # 4 token chunks per core
NO = O // 512          # 8 output column chunks
SCALE = 16.0 / np.sqrt(16.0)  # 4.0


def _build_nc():
    nc = bacc.Bacc(
        "TRN2", debug=False, target_bir_lowering=False, enable_partition_id=False
    )

    x = nc.dram_tensor("x", [TC, F], F32, kind="ExternalInput")
    # w1 packed: [128, KB*ER]; w1[p, k*ER + j] = Acat[j, k*128 + p]
    w1 = nc.dram_tensor("w1", [128, KB * ER], F32, kind="ExternalInput")
    # wr packed: [128, KB*E]; wr[p, k*E + e] = Wr[e, k*128 + p]  (exact fp32 router)
    wr = nc.dram_tensor("wr", [128, KB * E], F32, kind="ExternalInput")
    # btp: [64, O]; btp[e*R+r, o] = B[e, o, r] * SCALE
    bt = nc.dram_tensor("bt", [ER, O], F32, kind="ExternalInput")
    rp = nc.dram_tensor("rp", [E, ER], F32, kind="ExternalInput")
    out = nc.dram_tensor("out", [TC, O], F32, kind="ExternalOutput")
    dbg_lg = nc.dram_tensor("dbg_lg", [128, NT * E], F32, kind="ExternalOutput")
    dbg_g4 = nc.dram_tensor("dbg_g4", [128, NT * E], F32, kind="ExternalOutput")

    x_r = x.rearrange("(c p) f -> p c f", p=128)  # [128, NT, F]

    with tile.TileContext(nc) as tc:
        with (
            tc.tile_pool(name="consts", bufs=1) as consts,
            tc.tile_pool(name="xin", bufs=4) as xin_pool,
            tc.tile_pool(name="xt", bufs=KB) as xt_pool,
            tc.tile_pool(name="xtr", bufs=3) as xtr_pool,
            tc.tile_pool(name="gates", bufs=1) as gp,
            tc.tile_pool(name="outp", bufs=6) as outp,
            tc.tile_pool(name="ps_t", bufs=2, space="PSUM") as ps_t,
            tc.tile_pool(name="ps_mid", bufs=1, space="PSUM") as ps_mid,
            tc.tile_pool(name="ps_lg", bufs=1, space="PSUM") as ps_lg,
            tc.tile_pool(name="ps_g", bufs=1, space="PSUM") as ps_g,
            tc.tile_pool(name="ps_d", bufs=2, space="PSUM") as ps_d,
        ):
            # ---- constants / weights ----
            ident = consts.tile([128, 128], F32)
            masks.make_identity(nc, ident[:])

            w1raw = consts.tile([128, KB * ER], F32)
            nc.sync.dma_start(out=w1raw[:], in_=w1[:])
            w1s = consts.tile([128, KB * ER], F32R)
            nc.vector.tensor_copy(out=w1s[:], in_=w1raw[:])
            wrs = consts.tile([128, KB * E], F32)
            nc.sync.dma_start(out=wrs[:], in_=wr[:])
            btraw = consts.tile([ER, O], F32)
            nc.sync.dma_start(out=btraw[:], in_=bt[:])
            bts = consts.tile([ER, O], F32R)
            nc.vector.tensor_copy(out=bts[:], in_=btraw[:])

            # replication matrix [4, 64]: rep[e, e*16:(e+1)*16] = 1 (host-built)
            repm = consts.tile([E, ER], F32)
            nc.sync.dma_start(out=repm[:], in_=rp[:])

            # ---- phase 1: stream x, transpose, mm1 ----
            midps = ps_mid.tile([ER, TC], F32)  # [64, 512]
            lgps = ps_lg.tile([128, NT, E], F32)  # router logits [t, e] layout
            xts = []
            for k in range(KB):
                xcol = xin_pool.tile([128, NT, 128], F32)
                nc.sync.dma_start(out=xcol[:], in_=x_r[:, :, k * 128 : (k + 1) * 128])
                pst = ps_t.tile([128, TC], F32)
                for t in range(NT):
                    nc.tensor.transpose(
                        pst[:, t * 128 : (t + 1) * 128], xcol[:, t, :], ident[:]
                    )
                xt = xt_pool.tile([128, TC], F32, tag="xt_f32")
                nc.vector.tensor_copy(out=xt[:], in_=pst[:])
                xts.append(xt)
                xtr = xtr_pool.tile([128, TC], F32R, tag="xt_f32r")
                nc.vector.tensor_copy(out=xtr[:], in_=xt[:])
                nc.tensor.matmul(
                    midps[:],
                    w1s[:, k * ER : (k + 1) * ER],
                    xtr[:],
                    start=(k == 0),
                    stop=(k == KB - 1),
                )
            # router chains, k-contiguous per token chunk (interleaved fp32
            # accumulation chains are numerically broken on HW)
            for t in range(NT):
                for k in range(KB):
                    nc.tensor.matmul(
                        lgps[:, t, :],
                        xts[k][:, t * 128 : (t + 1) * 128],
                        wrs[:, k * E : (k + 1) * E],
                        start=(k == 0),
                        stop=(k == KB - 1),
                    )

            # ---- gates (logits already in [t, e] layout) ----
            lg4 = gp.tile([128, NT, E], F32, tag="lg4")
            nc.vector.tensor_copy(out=lg4[:], in_=lgps[:])
            nc.sync.dma_start(out=dbg_lg[:].rearrange("p (c e) -> p c e", e=E), in_=lg4[:])

            # top-2 softmax per chunk (free dim = 4 experts)
            g4 = gp.tile([128, NT, E], F32, tag="g4")
            for t in range(NT):
                lgc = lg4[:, t, :]
                nmx = gp.tile([128, 1], F32, tag="nmx")
                nc.vector.reduce_max(nmx[:], lgc, axis=mybir.AxisListType.X, negate=True)
                et = gp.tile([128, E], F32, tag="et")
                # exp(l - max): top-1 becomes exactly 1.0
                nc.scalar.activation(
                    et[:], lgc, mybir.ActivationFunctionType.Exp, bias=nmx[:], scale=1.0
                )
                m1 = gp.tile([128, E], F32, tag="m1")
                nc.vector.tensor_scalar(
                    out=m1[:], in0=et[:], scalar1=1.0, scalar2=0.0,
                    op0=mybir.AluOpType.is_ge,
                )
                t1 = gp.tile([128, E], F32, tag="t1")
                nc.vector.tensor_mul(t1[:], et[:], m1[:])
                et2 = gp.tile([128, E], F32, tag="et2")
                nc.vector.tensor_sub(et2[:], et[:], t1[:])  # et with top-1 zeroed
                mx2 = gp.tile([128, 1], F32, tag="mx2")
                nc.vector.reduce_max(mx2[:], et2[:], axis=mybir.AxisListType.X)
                m2 = gp.tile([128, E], F32, tag="m2")
                nc.vector.tensor_scalar(
                    out=m2[:], in0=et[:], scalar1=mx2[:], scalar2=0.0,
                    op0=mybir.AluOpType.is_ge,
                )
                em = gp.tile([128, E], F32, tag="em")
                nc.vector.tensor_mul(em[:], et[:], m2[:])
                z = gp.tile([128, 1], F32, tag="z")
                nc.vector.reduce_sum(z[:], em[:], axis=mybir.AxisListType.X)
                rz = gp.tile([128, 1], F32, tag="rz")
                nc.vector.reciprocal(rz[:], z[:])
                nc.vector.tensor_scalar(
                    out=g4[:, t, :], in0=em[:], scalar1=rz[:], scalar2=0.0,
                    op0=mybir.AluOpType.mult,
                )

            nc.sync.dma_start(out=dbg_g4[:].rearrange("p (c e) -> p c e", e=E), in_=g4[:])
            # transpose gates back -> [4e, 512t], then replicate rows 4->64
            psgt = ps_g.tile([E, TC], F32, tag="psgt")
            for t in range(NT):
                nc.tensor.transpose(
                    psgt[:, t * 128 : (t + 1) * 128], g4[:, t, :], ident[:]
                )
            gt = gp.tile([E, TC], F32, tag="gt")
            nc.vector.tensor_copy(out=gt[:], in_=psgt[:])
            psrep = ps_g.tile([ER, TC], F32, tag="psrep")
            nc.tensor.matmul(psrep[:], repm[:], gt[:], start=True, stop=True)
            grep = gp.tile([ER, TC], F32, tag="grep")
            nc.vector.tensor_copy(out=grep[:], in_=psrep[:])

            # midp = midT * gates_rep  [64, 512]
            midp = gp.tile([ER, TC], F32R, tag="midp")
            nc.vector.tensor_mul(midp[:], midps[:], grep[:])

            # ---- phase 2: mm2 + store ----
            for t in range(NT):
                for oc in range(NO):
                    pd = ps_d.tile([128, 512], F32)
                    nc.tensor.matmul(
                        pd[:],
                        midp[:, t * 128 : (t + 1) * 128],
                        bts[:, oc * 512 : (oc + 1) * 512],
                        start=True,
                        stop=True,
                    )
                    ob = outp.tile([128, 512], F32)
                    nc.any.tensor_copy(out=ob[:], in_=pd[:])
                    nc.sync.dma_start(
                        out=out[t * 128 : (t + 1) * 128, oc * 512 : (oc + 1) * 512],
                        in_=ob[:],
                    )
    nc.finalize()
    return nc


_NC_CACHE = None


def _get_nc():
    global _NC_CACHE
    if _NC_CACHE is None:
        _NC_CACHE = _build_nc()
    return _NC_CACHE


def _prep_weights(A, B, Wr):
    W1 = A.reshape(ER, F).astype(np.float32)
    # packed [128, KB*ER]: w1[p, k*ER+j] = W1[j, k*128+p]
    w1p = np.ascontiguousarray(
        W1.T.reshape(KB, 128, ER).transpose(1, 0, 2).reshape(128, KB * ER)
    )
    wrp = np.ascontiguousarray(
        Wr.T.reshape(KB, 128, E).transpose(1, 0, 2).reshape(128, KB * E)
    ).astype(np.float32)
    # btp [64, O] = B[e, o, r] * SCALE -> (e r) o
    btp = np.ascontiguousarray(B.transpose(0, 2, 1).reshape(ER, O) * SCALE).astype(
        np.float32
    )
    return w1p, wrp, btp


def kernel(x, A, B, Wr, _trace=False, _trace_kwargs=None):
    x = np.asarray(x, dtype=np.float32)
    A = np.asarray(A, dtype=np.float32)
    B = np.asarray(B, dtype=np.float32)
    Wr = np.asarray(Wr, dtype=np.float32)

    orig_shape = x.shape
    flat = np.ascontiguousarray(x.reshape(-1, orig_shape[-1]))
    w1p, wrp, btp = _prep_weights(A, B, Wr)
    repmat = np.zeros((E, ER), dtype=np.float32)
    for e in range(E):
        repmat[e, e * R : (e + 1) * R] = 1.0

    nc = _get_nc()
    in_maps = []
    for c in range(N_CORES):
        in_maps.append(
            {
                "x": np.ascontiguousarray(flat[c * TC : (c + 1) * TC, :]),
                "w1": w1p,
                "wr": wrp,
                "bt": btp,
                "rp": repmat,
            }
        )
    kw = {}
    if _trace:
        kw = dict(trace=True, trace_cores=[0], trace_kwargs=_trace_kwargs or {})
    res = run_bass_kernel_spmd(nc, in_maps, core_ids=list(range(N_CORES)), **kw)
    outs = [res.results[c]["out"] for c in range(N_CORES)]
    full = np.concatenate(outs, axis=0).reshape(*orig_shape[:-1], O)
    if _trace:
        kernel._last_results = res
    return full

